# revision 1
# baseline (speedup 1.0000x reference)
"""AttentionNeuronLayer Trainium2 kernel.

Strategy: the obs_dim rows of the LSTM state evolve fully independently
through the whole recurrence (each obs scalar is paired with the full
action vector); only the final `out = w @ s` contracts over obs rows, and
the final tanh is applied after that sum.  So we shard obs_dim 512 -> 64
rows per core across 8 cores with zero collectives: each core runs the
full T=256 recurrence on its 64 rows in a transposed layout
(pos_em/gate/msg dims on partitions, obs rows on the free dim), computes
its slice of w = tanh(q @ k.T / sqrt(d)) and the partial contraction
w.T @ s, and the host sums the 8 partial (T, hidden) results and applies
the final tanh.

Numerics: the task amplifies per-op rounding ~25x (256-step recurrence,
saturating attention); bf16 anywhere pushes the final error to ~5e-2.
The recurrence stays fp32; the off-recurrence attention operands (q, k,
tanh(w), s-at-out) are fp16 (1 cycle/row, col-tiling compatible, ~8e-3
final error — float32r would be more precise but its 4-byte weight load
cannot target col-offset PE tiles).

Per-core step layout (64 obs rows, steps processed in pairs for the
attention tail):
  gates.T (128 x [i,f,o,g]x64)  = W_hh_b @ hT_prev + W_ih1_b @ [s; a; 1]
    (g row pre-scaled by 2 so one Sigmoid op covers all four gates;
     tanh(g) = 2*sigmoid(2g) - 1 folded into the DVE c-update)
  kT (128 msg, 2x64 obs)        = Wk.T.T @ hT_pair (+ bk x ones)
  wT pair (128 = 2 steps x 64 obs, 1024 hidden) = kT_j.T @ qT, one tanh
  out pair (2, 1024) = blockdiag([s_t; s_t+1]).T @ wT_pair  (fp16),
    4 pairs accumulate into one col-tiled PSUM tile -> one DVE copy +
    one contiguous DMA per 8 steps.
  The attention tail (kT / w / out) is emitted 1-3 pairs behind the
  recurrence so every op enters its engine FIFO with dependencies
  already satisfied (engines are in-order; inline emission serializes
  the LSTM chain with the attention work).
"""

import sys

sys.path.insert(0, "/opt/trn_rl_repo")

import numpy as np

import concourse.bass as bass
import concourse.tile as tile
from concourse import mybir
from concourse.vector_clock import ScopedClock
from concourse.bass_utils import run_bass_kernel_spmd

OBS_DIM = 512
ACT_DIM = 32
HIDDEN_DIM = 1024
MSG_DIM = 128
POS_EM_DIM = 128
T = 256
NCORES = 8
SH = OBS_DIM // NCORES  # 64 obs rows per core

F32 = mybir.dt.float32
F16 = mybir.dt.float16
AF = mybir.ActivationFunctionType
ALU = mybir.AluOpType

# gate blocks laid out [i, f, g, o]: sigmoid(i,f,g) fires after three
# blocks' matmuls, sigmoid(o) trails off the critical path (g doubled)
_PERM = [0, 1, 2, 3]

TRACE = [False]  # test.py flips this for the profiled run
LAST_RESULTS = [None]


def _patched_drain_and_barrier(self, tick_clock, wait_clock):
    # This walrus build rejects instructions carrying more than one
    # sync-wait command; Tile's tail drain aggregates one wait per live
    # proc.  Re-emit the waits on individual single-wait NOPs instead.
    nc = self.nc
    carrier = nc.sync.nop(nofuse=True)
    wait_clock.add_sem_waits(carrier.ins, ScopedClock({None: tick_clock.global_clock}))
    si = carrier.ins.sync_info
    waits = list(si.on_wait) if si is not None and si.on_wait else []
    if si is not None:
        carrier.ins.sync_info = mybir.SyncInfo(
            on_wait=[], on_update=list(si.on_update or [])
        )
    for w in waits:
        n2 = nc.sync.nop(nofuse=True)
        n2.ins.sync_info = mybir.SyncInfo(on_wait=[w], on_update=[])
    nc.sync.drain()
    nc.all_engine_barrier()
    popped = nc._tile_sem_poison_stack.pop()
    assert popped is self._sem_poison
    nc.clear_and_free_semaphores(list(self.sems.allocated().values()))
    nc.all_engine_barrier()


tile.TileContext._drain_and_barrier = _patched_drain_and_barrier


def _split_multi_waits(module):
    """This walrus build accepts at most one sync-wait command per
    instruction.  Move excess waits onto same-engine NoOps inserted just
    before the instruction — the engine stream is serial, so gating an
    earlier NoOp on the same conditions is equivalent (DMA triggers are
    issued by their engine in program order, so this holds for DMACopy
    too)."""
    import copy as _copy

    counter = [0]
    new_module = _copy.replace(module, functions=[])
    for function in module.functions:
        new_function = _copy.replace(function, blocks=[])
        new_function.set_allocations_from_list(function.allocations)
        for block in function.blocks:
            new_insts = []
            for inst in block.instructions:
                si = inst.sync_info
                waits = list(si.on_wait) if si is not None and si.on_wait else []
                if len(waits) > 1:
                    for w in waits[:-1]:
                        counter[0] += 1
                        nop = mybir.InstNoOp(
                            engine=inst.engine, name=f"I-ws{counter[0]}"
                        )
                        nop.sync_info = mybir.SyncInfo(on_wait=[w], on_update=[])
                        new_insts.append(nop)
                    inst.sync_info = mybir.SyncInfo(
                        on_wait=[waits[-1]], on_update=list(si.on_update or [])
                    )
                new_insts.append(inst)
            new_function.blocks.append(_copy.replace(block, instructions=new_insts))
        new_module.functions.append(new_function)
    return new_module


_NC_CACHE = {}


def _build_nc(split=True):
    if split in _NC_CACHE:
        return _NC_CACHE[split]
    nc = bass.Bass()
    whhT = nc.declare_dram_parameter("whhT", [POS_EM_DIM, 512], F16, isOutput=False)
    wih3T = nc.declare_dram_parameter("wih3T", [102, 512], F16, isOutput=False)
    wkT = nc.declare_dram_parameter("wkT", [POS_EM_DIM, MSG_DIM], F32, isOutput=False)
    bkr = nc.declare_dram_parameter("bkr", [1, MSG_DIM], F32, isOutput=False)
    qT = nc.declare_dram_parameter("qT", [MSG_DIM, HIDDEN_DIM], F16, isOutput=False)
    xa3 = nc.declare_dram_parameter("xa3", [102, T * SH], F16, isOutput=False)
    xTp = nc.declare_dram_parameter("xTp", [2 * SH, T], F16, isOutput=False)
    # transposed out accumulation: 64 steps (32 pairs x 8 hidden blocks x
    # N=2) fill one (128, 512) PSUM bank; host decodes the layout
    outs = nc.declare_dram_parameter("outs", [T // 64, 128, 512], F32, isOutput=True)

    inv_scale = 1.0 / float(np.sqrt(np.float32(MSG_DIM)))

    with tile.TileContext(nc) as tc:
        with (
            tc.tile_pool(name="const", bufs=1) as const,
            tc.tile_pool(name="state", bufs=1) as state,
            tc.tile_pool(name="hs", bufs=5) as hsp,
            tc.tile_pool(name="work", bufs=3) as work,
            tc.tile_pool(name="wap", bufs=4) as wap,
            tc.tile_pool(name="stg", bufs=2) as stg,
            tc.tile_pool(name="pg", bufs=1, space="PSUM") as pgp,
            tc.tile_pool(name="pk", bufs=1, space="PSUM") as pkp,
            tc.tile_pool(name="pw", bufs=2, space="PSUM") as pwp,
            tc.tile_pool(name="po", bufs=1, space="PSUM") as pop,
        ):
            whhT_sb = const.tile([POS_EM_DIM, 512], F16)
            wih3T_sb = const.tile([102, 512], F16)
            wkT_sb = const.tile([POS_EM_DIM, MSG_DIM], F32)
            bkr_sb = const.tile([1, MSG_DIM], F32)
            qT_sb = const.tile([MSG_DIM, HIDDEN_DIM], F16)
            xa3_sb = const.tile([102, T * SH], F16)
            xTp_sb = const.tile([2 * SH, T], F16)
            for dst, src in (
                (whhT_sb, whhT),
                (wih3T_sb, wih3T),
                (wkT_sb, wkT),
                (bkr_sb, bkr),
                (qT_sb, qT),
                (xa3_sb, xa3),
                (xTp_sb, xTp),
            ):
                nc.sync.dma_start(out=dst[:], in_=src[:])

            cT = state.tile([POS_EM_DIM, SH], F32)
            h0 = const.tile([POS_EM_DIM, SH], F16)
            ones_sb = const.tile([1, 2 * SH], F32)
            nc.vector.memset(cT[:], 0.0)
            nc.vector.memset(h0[:], 0.0)
            nc.vector.memset(ones_sb[:], 1.0)

            poT = pop.tile([128, 512], F32)
            nc.vector.memset(poT[:], 0.0)

            hbufs = {}
            h16bufs = {}
            kbufs = {}
            wabufs = {}

            def emit_kT(p):
                pk = pkp.tile([MSG_DIM, 2 * SH], F32, tag="pk")
                nc.tensor.matmul(
                    pk[:], wkT_sb[:], hbufs[p][:], start=True, stop=False
                )
                nc.tensor.matmul(pk[:], bkr_sb[:], ones_sb[:], start=False, stop=True)
                kTp = work.tile([MSG_DIM, 2 * SH], F16, name=f"kTp{p}", tag="kTp")
                nc.vector.tensor_copy(kTp[:], pk[:])
                kbufs[p] = kTp

            def emit_w(p):
                # rows 0-63 = even step of the pair, 64-127 = odd step
                kTp = kbufs.pop(p)
                pw = pwp.tile([128, HIDDEN_DIM], F32, name=f"pw{p}", tag="pw")
                for jj in range(2):
                    tp = None if jj == 0 else (0, 64)
                    for h2 in range(2):
                        nc.tensor.matmul(
                            pw[64 * jj : 64 * jj + 64, 512 * h2 : 512 * h2 + 512],
                            kTp[:, SH * jj : SH * jj + SH],
                            qT_sb[:, 512 * h2 : 512 * h2 + 512],
                            start=True,
                            stop=True,
                            tile_position=tp,
                        )
                wa = wap.tile([128, HIDDEN_DIM], F16, name=f"wa{p}", tag="wa")
                nc.scalar.activation(wa[:], pw[:], AF.Tanh, scale=inv_scale)
                wabufs[p] = wa

            def emit_out(p):
                # transposed: out rows = hidden sub-dim (M=128), col pair =
                # the two steps; 32 pairs accumulate in one PSUM bank
                wa = wabufs.pop(p)
                base = 16 * (p % 32)
                for i in range(8):
                    nc.tensor.matmul(
                        poT[:, base + 2 * i : base + 2 * i + 2],
                        wa[:, 128 * i : 128 * i + 128],
                        xTp_sb[:, 2 * p : 2 * p + 2],
                        start=True,
                        stop=True,
                    )
                if p % 32 == 31:
                    so = stg.tile([128, 512], F32, tag="so")
                    nc.scalar.copy(out=so[:], in_=poT[:])
                    nc.sync.dma_start(out=outs[p // 32, :, :], in_=so[:])

            for t in range(T):
                j = t % 2  # position within the step pair
                p = t // 2
                if j == 0:
                    hbufs[p] = hsp.tile(
                        [POS_EM_DIM, 2 * SH], F32, name=f"hTp{p}", tag="hTp"
                    )
                    h16bufs[p] = hsp.tile(
                        [POS_EM_DIM, 2 * SH], F16, name=f"h16p{p}", tag="h16p"
                    )
                hTp = hbufs[p]
                hT = hTp[:, SH * j : SH * j + SH]
                # gates read the fp16 mirror of h (4x faster PE stream);
                # the k-path keeps the fp32 original
                h_prev = (
                    h0[:]
                    if t == 0
                    else (
                        h16bufs[p - 1][:, SH:] if j == 0 else h16bufs[p][:, 0:SH]
                    )
                )
                # ---- gates ----
                pg = pgp.tile([128, 256], F32, tag="pg")
                for b in range(4):
                    nc.tensor.matmul(
                        pg[:, 64 * b : 64 * b + 64],
                        whhT_sb[:, 128 * b : 128 * b + 128],
                        h_prev,
                        start=True,
                        stop=False,
                    )
                    nc.tensor.matmul(
                        pg[:, 64 * b : 64 * b + 64],
                        wih3T_sb[:, 128 * b : 128 * b + 128],
                        xa3_sb[:, SH * t : SH * t + SH],
                        start=False,
                        stop=True,
                    )
                sig = work.tile([128, 256], F32, tag="sig")
                nc.scalar.activation(sig[:, 0:192], pg[:, 0:192], AF.Sigmoid)
                nc.scalar.activation(sig[:, 192:256], pg[:, 192:256], AF.Sigmoid)
                # ---- c, h ----  (tanh(g) = 2*sigmoid(2g) - 1)
                t1 = work.tile([128, SH], F32, tag="t1")
                nc.vector.tensor_mul(t1[:], sig[:, 0:64], sig[:, 128:192])
                nc.vector.scalar_tensor_tensor(
                    t1[:], t1[:], 2.0, sig[:, 0:64], ALU.mult, ALU.subtract
                )
                nc.vector.tensor_mul(cT[:], sig[:, 64:128], cT[:])
                nc.vector.tensor_add(cT[:], cT[:], t1[:])
                tct = work.tile([128, SH], F32, tag="tct")
                nc.scalar.activation(tct[:], cT[:], AF.Tanh)
                # fp16 mirror first: the next step's gate matmuls wait
                # only on this op; the fp32 copy (k-path) trails off-chain
                nc.vector.tensor_mul(
                    h16bufs[p][:, SH * j : SH * j + SH], sig[:, 192:256], tct[:]
                )
                nc.vector.tensor_mul(hT, sig[:, 192:256], tct[:])
                # ---- pipelined attention tail: kT lags 1 pair, w lags 2,
                # out lags 3 — ops enter their engine FIFOs with deps
                # already satisfied ----
                if j == 1 and p >= 1:
                    emit_kT(p - 1)
                    if p >= 2:
                        emit_w(p - 2)
                    if p >= 3:
                        emit_out(p - 3)
                    hbufs.pop(p - 3, None)
                    h16bufs.pop(p - 3, None)
            last = T // 2 - 1  # 127
            emit_kT(last)
            emit_w(last - 1)
            emit_w(last)
            emit_out(last - 2)
            emit_out(last - 1)
            emit_out(last)
    if split:
        nc.m = _split_multi_waits(nc.m)
    _NC_CACHE[split] = nc
    return nc


def kernel(
    obs,
    prev_act,
    in_shift,
    in_scale,
    pos_embedding,
    W_ih,
    b_ih,
    W_hh,
    b_hh,
    Wq,
    bq,
    Wk,
    bk,
):
    obs = np.asarray(obs, np.float32)
    prev_act = np.asarray(prev_act, np.float32)
    in_shift = np.asarray(in_shift, np.float32)
    in_scale = np.asarray(in_scale, np.float32)
    pos_embedding = np.asarray(pos_embedding, np.float32)
    W_ih = np.asarray(W_ih, np.float32)
    b_ih = np.asarray(b_ih, np.float32)
    W_hh = np.asarray(W_hh, np.float32)
    b_hh = np.asarray(b_hh, np.float32)
    Wq = np.asarray(Wq, np.float32)
    bq = np.asarray(bq, np.float32)
    Wk = np.asarray(Wk, np.float32)
    bk = np.asarray(bk, np.float32)

    x = (obs - in_shift) / (in_scale + 1e-8)  # (T, 512)
    q = pos_embedding @ Wq.T + bq  # (1024, 128)
    qT = np.ascontiguousarray(q.T)  # (128, 1024)

    def blocks(mat_rows):  # reorder gate blocks to [i, f, o, g]
        return np.concatenate([mat_rows[128 * p : 128 * p + 128] for p in _PERM], 0)

    W_ih_r = blocks(W_ih)  # (512, 33)
    W_hh_r = blocks(W_hh)  # (512, 128)
    b_r = blocks((b_ih + b_hh)[:, None])[:, 0]  # (512,)
    # g block (cols 384:512 after reorder) doubled: tanh(g) = 2*sig(2g)-1
    gs = np.ones((512, 1), np.float32)
    gs[256:384] = 2.0
    W_ih_r = W_ih_r * gs
    W_hh_r = W_hh_r * gs
    b_r = b_r * gs[:, 0]

    whhT = np.ascontiguousarray(W_hh_r.T)  # (128, 512)
    wih1T = np.concatenate(
        [W_ih_r[:, 0:1].T, np.ascontiguousarray(W_ih_r[:, 1:33].T), b_r[None, :]], 0
    )  # (34, 512)
    # fp16 hi/lo folded into K: [Whi; Whi; Wlo] x [xhi; xlo; xhi] gives
    # Whi*xhi + Whi*xlo + Wlo*xhi (residual ~2^-22) in one K=102 fp16 MM
    whi = wih1T.astype(np.float16)
    wlo = (wih1T - whi.astype(np.float32)).astype(np.float16)
    wih3T = np.concatenate([whi, whi, wlo], 0)  # (102, 512) fp16
    wkT = np.ascontiguousarray(Wk.T)  # (128, 128)
    bkr = bk[None, :]  # (1, 128)

    nc = _build_nc()
    shared = {
        "whhT": whhT.astype(np.float16),
        "wih3T": np.ascontiguousarray(wih3T),
        "wkT": wkT,
        "bkr": np.ascontiguousarray(bkr),
        "qT": qT.astype(np.float16),
    }
    in_maps = []
    for c in range(NCORES):
        xs = x[:, c * SH : (c + 1) * SH]  # (T, 64)
        xa1 = np.empty((34, T * SH), np.float32)
        xa1[0] = xs.reshape(-1)
        xa1[1:33] = np.repeat(prev_act.T, SH, axis=1).reshape(32, T * SH)
        xa1[33] = 1.0
        xahi = xa1.astype(np.float16)
        xalo = (xa1 - xahi.astype(np.float32)).astype(np.float16)
        xa3 = np.concatenate([xahi, xalo, xahi], 0)  # (102, T*SH) fp16
        # block-diagonal paired s columns: col t has s_t in rows [64j, 64j+64)
        # for j = t%2, zeros elsewhere
        xTp = np.zeros((2 * SH, T), np.float16)
        xTp[0:SH, 0::2] = xs.T[:, 0::2]
        xTp[SH : 2 * SH, 1::2] = xs.T[:, 1::2]
        in_maps.append({**shared, "xa3": xa3, "xTp": xTp})

    res = run_bass_kernel_spmd(nc, in_maps, list(range(NCORES)), trace=TRACE[0])
    LAST_RESULTS[0] = res
    total = np.zeros((T, HIDDEN_DIM), np.float32)
    for c in range(NCORES):
        raw = res.results[c]["outs"]  # (T//64, 128, 512)
        # col = (pair%32)*16 + hidden_block*2 + step_in_pair
        total += np.transpose(
            raw.reshape(T // 64, 128, 32, 8, 2), (0, 2, 4, 3, 1)
        ).reshape(T, HIDDEN_DIM)
    return np.tanh(total).astype(np.float32)

